# revision 1
# baseline (speedup 1.0000x reference)
"""Trainium2 Bass kernel: 3x3 VALID conv (NCHW/OIHW) + bias + /2 + LeakyReLU.

Full-input contract: kernel(x, weight, bias) takes the complete arrays,
shards the batch dim across 8 NeuronCores (2 images per core), runs the
Bass program SPMD, and concatenates the per-core outputs.

Compute strategy (per core, per image):
  - SBUF layout: input row h, channel c -> partition 32*(h%4)+c, free
    offset (h//4)*258 + w  (rows padded 256->258 so the kw=1,2 taps can
    read a full 256-wide window without crossing rows).
  - Each output row o needs input rows o..o+2, which land in 3 distinct
    32-partition groups -> the 3 kh-taps run as concurrent 32x32 PE
    sub-tiles (tile_position row groups). 4 output rows are processed per
    round in the 4 PSUM column groups -> 12 concurrent sub-tiles.
  - kw taps are free-dim offsets (0/1/2) into the same SBUF row.
  - bf16 compute; the SWDGE input DMAs cast f32->bf16 in flight (free).
  - Each kh tap accumulates in its own PSUM plane (a region may only be
    written by one tile position); planes rotate over all 8 PSUM banks
    for eviction-chain pipelining. Eviction: ACT copy + 2 DVE adds +
    one ScalarE Lrelu (out = Lrelu(sum*0.5 + b/2), alpha=0.01) into an
    SBUF staging tile DMA'd out in 32-row batches.
"""

import sys

if "/opt/trn_rl_repo" not in sys.path:
    sys.path.insert(0, "/opt/trn_rl_repo")

import numpy as np

import concourse.bass as bass
import concourse.tile as tile
from concourse import bacc
from concourse import mybir
from concourse.bass_utils import run_bass_kernel_spmd

N_CORES = 8
IMGS_PER_CORE = 2
C = 32
H = 256
W = 256
OH = 254
OW = 254
G = 4            # partition groups = h mod 4
HD = H // G      # 64 rows per group
WPAD = W + 2     # per-row pad so kw shifts stay in-row
NFREE = 256      # matmul free dim (>=256 keeps float32r at full rate)
F32 = mybir.dt.float32
F32R = mybir.dt.float32r
BF16 = mybir.dt.bfloat16
LRELU = mybir.ActivationFunctionType.Lrelu


def build_nc(repeat=1):
    nc = bacc.Bacc()
    x_ext = nc.declare_dram_parameter(
        "x", [IMGS_PER_CORE, C, H, W], F32, isOutput=False
    )
    # host-prepared: wr[32g+k, tap, m] = weight[m, k, kh, kw]; biasr = bias/2 tiled 4x
    w_ext = nc.declare_dram_parameter("wr", [128, 9, C], BF16, isOutput=False)
    b_ext = nc.declare_dram_parameter("biasr", [128], F32, isOutput=False)
    y_ext = nc.declare_dram_parameter(
        "y", [IMGS_PER_CORE, C, OH, OW], F32, isOutput=True
    )

    with tile.TileContext(nc) as tc:
        with (
            tc.tile_pool(name="xp", bufs=2) as xpool,
            tc.tile_pool(name="const", bufs=1) as cpool,
            tc.tile_pool(name="ps", bufs=1, space="PSUM") as pspool,
            tc.tile_pool(name="ev", bufs=6) as evpool,
            tc.tile_pool(name="outp", bufs=3) as opool,
        ):
            # Weights: partition 32g+k (k = c_in), free (tap, m = c_out),
            # replicated into all 4 partition groups so lhsT.base_partition
            # matches the rhs row group (tile_position auto-derivation).
            w_sb = cpool.tile([128, 9, C], BF16)
            nc.sync.dma_start(out=w_sb, in_=w_ext[:])

            bias_half = cpool.tile([128, 1], F32)
            nc.sync.dma_start(out=bias_half, in_=b_ext[:].unsqueeze(1))


            bank_ctr = [0]
            for img_rep in range(IMGS_PER_CORE * repeat):
                img = img_rep % IMGS_PER_CORE
                x_sb = xpool.tile([128, HD, WPAD], BF16)
                nc.vector.memset(x_sb[:, :, W:WPAD], 0.0)
                # h = hd*4 + hm  ->  partition group hm, free row hd
                # SWDGE dma casts f32 -> bf16 in flight
                xsrc = x_ext[:][img].rearrange("c (hd hm) w -> hm c hd w", hm=G)
                # halves let round 0 start after ~4MB instead of 8MB
                for half in range(2):
                    hd0, hd1 = 32 * half, 32 * (half + 1)
                    for g in range(G):
                        nc.gpsimd.dma_start(
                            out=x_sb[32 * g : 32 * (g + 1), hd0:hd1, 0:W],
                            in_=xsrc[g][:, hd0:hd1, :],
                        )

                for b in range(8):  # batches of up to 32 output rows
                    rows0 = 32 * b
                    nrounds = min(8, (OH - rows0 + 3) // 4)
                    stage = opool.tile([128, 8, NFREE], F32)
                    for rb in range(nrounds):
                        h0 = rows0 + 4 * rb
                        njs = min(4, OH - h0)
                        # one PSUM plane per kh: each [32,256] region is
                        # written by exactly one PE tile position (multi-
                        # row-group accumulation into one region faults).
                        # rotate the 3 planes across all 8 PSUM banks for
                        # ~2.7 rounds of eviction-chain pipelining.
                        c0 = bank_ctr[0]
                        bank_ctr[0] += 3
                        pl0 = pspool.tile([128, NFREE], F32, tag=f"bk{c0 % 8}")
                        pl1 = pspool.tile(
                            [128, NFREE], F32, tag=f"bk{(c0 + 1) % 8}"
                        )
                        pl2 = pspool.tile(
                            [128, NFREE], F32, tag=f"bk{(c0 + 2) % 8}"
                        )
                        planes = [pl0, pl1, pl2]
                        for j in range(njs):
                            o = h0 + j
                            for kh in range(3):
                                rho = o + kh
                                g = rho % 4
                                hd = rho // 4
                                for kw in range(3):
                                    nc.tensor.matmul(
                                        planes[kh][32 * j : 32 * (j + 1), :],
                                        w_sb[
                                            32 * g : 32 * (g + 1),
                                            kh * 3 + kw,
                                            :,
                                        ],
                                        x_sb[
                                            32 * g : 32 * (g + 1),
                                            hd,
                                            kw : kw + NFREE,
                                        ],
                                        start=(kw == 0),
                                        stop=(kw == 2),
                                        tile_position=(32 * g, 32 * j),
                                    )
                        np_used = 32 * njs
                        a_sb = evpool.tile([128, NFREE], F32, tag="a")
                        a2_sb = evpool.tile([128, NFREE], F32, tag="a2")
                        b_sb = evpool.tile([128, NFREE], F32, tag="b")
                        nc.scalar.activation(
                            out=a_sb[0:np_used],
                            in_=pl0[0:np_used],
                            func=mybir.ActivationFunctionType.Copy,
                            bias=0.0,
                            scale=1.0,
                        )
                        nc.vector.tensor_add(
                            a2_sb[0:np_used], a_sb[0:np_used], pl1[0:np_used]
                        )
                        nc.vector.tensor_add(
                            b_sb[0:np_used], a2_sb[0:np_used], pl2[0:np_used]
                        )
                        nc.scalar.activation(
                            out=stage[0:np_used, rb, :],
                            in_=b_sb[0:np_used],
                            func=LRELU,
                            bias=bias_half[0:np_used],
                            scale=0.5,
                            alpha=0.01,
                        )
                    # store: per column group j, rows rows0+4*rb+j (stride 4)
                    if True:
                        for j in range(4):
                            nrb_j = 0
                            while nrb_j < nrounds and rows0 + 4 * nrb_j + j < OH:
                                nrb_j += 1
                            if nrb_j == 0:
                                continue
                            src = stage[32 * j : 32 * (j + 1), 0:nrb_j, 0:OW]
                            dst = y_ext[:][img][
                                :,
                                rows0 + j : min(rows0 + j + 4 * nrb_j, OH) : 4,
                                :,
                            ]
                            nc.sync.dma_start(out=dst, in_=src)
    nc.compile()
    return nc


_CACHE = {}


def _get_nc(repeat=1):
    key = f"nc{repeat}"
    if key not in _CACHE:
        _CACHE[key] = build_nc(repeat)
    return _CACHE[key]


def kernel(x, weight, bias):
    x = np.ascontiguousarray(np.asarray(x, dtype=np.float32))
    weight = np.asarray(weight, dtype=np.float32)
    bias = np.asarray(bias, dtype=np.float32)
    # wr[32g+k, tap, m] = weight[m, k, kh, kw], replicated into 4 groups
    import ml_dtypes
    wr = np.ascontiguousarray(
        np.tile(
            np.transpose(weight, (1, 2, 3, 0)).reshape(C, 9, C), (G, 1, 1)
        ).astype(ml_dtypes.bfloat16)
    )
    biasr = np.ascontiguousarray(np.tile(bias * 0.5, G))
    nc = _get_nc()
    in_maps = [
        {
            "x": x[IMGS_PER_CORE * i : IMGS_PER_CORE * (i + 1)],
            "wr": wr,
            "biasr": biasr,
        }
        for i in range(N_CORES)
    ]
    try:
        res = run_bass_kernel_spmd(nc, in_maps, core_ids=list(range(N_CORES)))
    except Exception:
        # transient device fault (axon terminal resets itself in ~2 min)
        import time as _time

        _time.sleep(130)
        res = run_bass_kernel_spmd(nc, in_maps, core_ids=list(range(N_CORES)))
    return np.concatenate([res.results[i]["y"] for i in range(N_CORES)], axis=0)



# revision 8
# speedup vs baseline: 2.8619x; 2.8619x over previous
"""Trainium2 Bass kernel: 3x3 VALID conv (NCHW/OIHW) + bias + /2 + LeakyReLU.

Full-input contract: kernel(x, weight, bias) takes the complete arrays,
shards the batch dim across 8 NeuronCores (2 images per core), runs the
Bass program SPMD, and concatenates the per-core outputs.

v3 strategy (per core, per image):
  - Host pre-casts x to bf16 and pre-arranges it into the exact SBUF
    layout as two row-phase copies, so each copy loads with ONE
    128-partition DMA (all DMAs span 128 partitions; cost scales with
    per-partition bytes):
      copy A: xr[0, img, 32q+c, r, w] = x[c, 4r+q,   w]
      copy B: xr[1, img, 32q+c, r, w] = x[c, 4r+q+2, w]  (zero-pad tail)
  - kh-stacked contraction: one matmul covers all 3 kh taps. All matmuls
    are K=128 with weight variant v=s%2 whose unused 32-partition block is
    zero, so output row o = 64j+8t+s uses rhs = copy[s<2? A:B][:, r, ...],
    r = 16j+2t, lhsT = wr[:, v, kw, :]; 3 matmuls (kw taps) accumulate
    into one PSUM plane. N=254 per matmul.
  - Output in 4 vertical bands; band j accumulates in PSUM partitions
    32j..32j+31. y is declared banded+padded [4, 32, 64, 254] per image
    (rows 254/255 junk), so each 8-row group stores with ONE 128-partition
    DMA; the host reassembles/strips.
  - Epilogue: one ScalarE Lrelu per round, out = Lrelu(psum*0.5 + b/2),
    bf16 into a dst-linear stage tile.
"""

import sys

if "/opt/trn_rl_repo" not in sys.path:
    sys.path.insert(0, "/opt/trn_rl_repo")

import numpy as np

import concourse.bass as bass
import concourse.tile as tile
from concourse import bacc
from concourse import mybir
from concourse.bass_utils import run_bass_kernel_spmd

N_CORES = 8
IMGS_PER_CORE = 2
C = 32
H = 256
W = 256
OH = 254
OW = 254
F32 = mybir.dt.float32
BF16 = mybir.dt.bfloat16
LRELU = mybir.ActivationFunctionType.Lrelu


def build_nc(repeat=1):
    nc = bacc.Bacc()
    x_ext = nc.declare_dram_parameter(
        "xr", [2, IMGS_PER_CORE, 128, 64, W], BF16, isOutput=False
    )
    # wr[p, v, kw, m]: v=0 -> rows 0..95 hold (s=p//32, c=p%32) tap weights;
    # v=1 -> rows 32..127 hold (s=p//32-1, c=p%32). Unused rows zero.
    w_ext = nc.declare_dram_parameter("wr", [128, 2, 3, C], BF16, isOutput=False)
    b_ext = nc.declare_dram_parameter("biasr", [128], F32, isOutput=False)
    # banded, padded output: y[img, j, m, hb, w] = out[m, 64j+hb, w]
    y_ext = nc.declare_dram_parameter(
        "y", [IMGS_PER_CORE, 4, C, 64, OW], BF16, isOutput=True
    )

    with tile.TileContext(nc) as tc:
        with (
            tc.tile_pool(name="xa", bufs=2) as xapool,
            tc.tile_pool(name="xbp", bufs=2) as xbpool,
            tc.tile_pool(name="const", bufs=1) as cpool,
            tc.tile_pool(name="ps", bufs=1, space="PSUM") as pspool,
            tc.tile_pool(name="outp", bufs=3) as opool,
        ):
            w_sb = cpool.tile([128, 2, 3, C], BF16)
            nc.sync.dma_start(out=w_sb, in_=w_ext[:])
            bias_half = cpool.tile([128, 1], F32)
            nc.sync.dma_start(out=bias_half, in_=b_ext[:].unsqueeze(1))

            bank_ctr = [0]
            for img_rep in range(IMGS_PER_CORE * repeat):
                img = img_rep % IMGS_PER_CORE
                xa = xapool.tile([128, 64, W], BF16)
                xb = xbpool.tile([128, 64, W], BF16)
                nc.sync.dma_start(out=xa, in_=x_ext[:][0][img])
                nc.sync.dma_start(out=xb, in_=x_ext[:][1][img])

                ydst = y_ext[:][img].rearrange("j m hb w -> (j m) hb w")
                for t in range(8):  # 8-row groups per band
                    stage = opool.tile([128, 8, OW], BF16)
                    if t == 7:
                        # band-3 rows 254/255 are junk pad; define them
                        nc.vector.memset(stage[96:128, 6:8, :], 0.0)
                    for s in range(4):
                        # band j: rows o = 64j+8t+s and o+4 (h2 slot)
                        v = s % 2
                        xt = xa if s < 2 else xb
                        plane = pspool.tile(
                            [128, 2, 256], F32, tag=f"bk{bank_ctr[0] % 8}"
                        )
                        bank_ctr[0] += 1
                        edge = t == 7 and s >= 2  # rows 254/255 absent
                        for j in range(4):
                            r = 16 * j + 2 * t
                            if edge and j == 3:
                                out_ap = plane[96:128, 0, 0:OW]
                                for kw in range(3):
                                    nc.tensor.matmul(
                                        out_ap,
                                        w_sb[:, v, kw, :],
                                        xt[:, r, kw : kw + OW],
                                        start=(kw == 0),
                                        stop=(kw == 2),
                                        tile_position=(0, 96),
                                    )
                            else:
                                for h2 in range(2):
                                    out_ap = plane[
                                        32 * j : 32 * (j + 1), h2, 0:OW
                                    ]
                                    for kw in range(3):
                                        nc.tensor.matmul(
                                            out_ap,
                                            w_sb[:, v, kw, :],
                                            xt[:, r + h2, kw : kw + OW],
                                            start=(kw == 0),
                                            stop=(kw == 2),
                                            tile_position=(0, 32 * j),
                                        )
                        # evict: stage slot (s + 4*h2) = band row (8t+s+4*h2)
                        if edge:
                            nc.scalar.activation(
                                out=stage[0:96, s : s + 5 : 4, :],
                                in_=plane[0:96, :, 0:OW],
                                func=LRELU,
                                bias=bias_half[0:96],
                                scale=0.5,
                                alpha=0.01,
                            )
                            nc.scalar.activation(
                                out=stage[96:128, s, :],
                                in_=plane[96:128, 0, 0:OW],
                                func=LRELU,
                                bias=bias_half[96:128],
                                scale=0.5,
                                alpha=0.01,
                            )
                        else:
                            nc.scalar.activation(
                                out=stage[:, s : s + 5 : 4, :],
                                in_=plane[:, :, 0:OW],
                                func=LRELU,
                                bias=bias_half,
                                scale=0.5,
                                alpha=0.01,
                            )
                    nc.sync.dma_start(
                        out=ydst[:, 8 * t : 8 * t + 8, :], in_=stage
                    )
    nc.compile()
    return nc


_CACHE = {}


def _get_nc(repeat=1):
    key = f"nc{repeat}"
    if key not in _CACHE:
        _CACHE[key] = build_nc(repeat)
    return _CACHE[key]


def make_wr_biasr(weight, bias):
    import ml_dtypes

    weight = np.asarray(weight, dtype=np.float32)
    bias = np.asarray(bias, dtype=np.float32)
    wr = np.zeros((128, 2, 3, C), dtype=np.float32)
    for p in range(128):
        s0, c0 = p // 32, p % 32
        if s0 < 3:  # v=0: rows 0..95
            wr[p, 0, :, :] = weight[:, c0, s0, :].T  # [kw, m]
        if 1 <= s0 <= 3:  # v=1: rows 32..127, s = p//32 - 1
            wr[p, 1, :, :] = weight[:, c0, s0 - 1, :].T
    wr = np.ascontiguousarray(wr.astype(ml_dtypes.bfloat16))
    biasr = np.ascontiguousarray(np.tile(bias * 0.5, 4).astype(np.float32))
    return wr, biasr


def make_xr(x):
    """xr[copy, n, 32q+c, r, w]: copy 0 holds x rows 4r+q, copy 1 rows 4r+q+2."""
    import ml_dtypes

    x = np.asarray(x, dtype=np.float32).astype(ml_dtypes.bfloat16)
    n = x.shape[0]
    x2 = np.concatenate(
        [x[:, :, 2:, :], np.zeros((n, C, 2, W), dtype=x.dtype)], axis=2
    )
    out = np.empty((2, n, 128, 64, W), dtype=x.dtype)
    for ci, src in enumerate((x, x2)):
        # [n, c, 64r, 4q, w] -> [n, q, c, r, w] -> [n, 128, 64, w]
        v = src.reshape(n, C, 64, 4, W).transpose(0, 3, 1, 2, 4)
        out[ci] = v.reshape(n, 128, 64, W)
    return np.ascontiguousarray(out)


def make_in_maps(x, weight, bias):
    xr = make_xr(x)
    wr, biasr = make_wr_biasr(weight, bias)
    return [
        {
            "xr": xr[:, IMGS_PER_CORE * i : IMGS_PER_CORE * (i + 1)],
            "wr": wr,
            "biasr": biasr,
        }
        for i in range(N_CORES)
    ]


def postprocess_y(y_banded):
    """[n, 4, C, 64, OW] bf16 -> [n, C, 254, 254] f32."""
    y = np.asarray(y_banded).astype(np.float32)
    n = y.shape[0]
    y = y.transpose(0, 2, 1, 3, 4).reshape(n, C, 256, OW)
    return np.ascontiguousarray(y[:, :, 0:OH, :])


def kernel(x, weight, bias):
    in_maps = make_in_maps(x, weight, bias)
    nc = _get_nc()
    try:
        res = run_bass_kernel_spmd(nc, in_maps, core_ids=list(range(N_CORES)))
    except Exception:
        # transient device fault (axon terminal resets itself in ~2 min)
        import time as _time

        _time.sleep(130)
        res = run_bass_kernel_spmd(nc, in_maps, core_ids=list(range(N_CORES)))
    y = np.concatenate(
        [postprocess_y(res.results[i]["y"]) for i in range(N_CORES)], axis=0
    )
    return y


# revision 9
# speedup vs baseline: 3.0174x; 1.0543x over previous
"""v4: like v3 but each image is processed in NS=4 row-splits with chunked
A/B loads, so compute starts after one small chunk DMA instead of the whole
image. Bands shrink to 16 output rows; y is declared [img, split, band, C,
16, OW]."""

import sys

if "/opt/trn_rl_repo" not in sys.path:
    sys.path.insert(0, "/opt/trn_rl_repo")

import numpy as np

import concourse.bass as bass
import concourse.tile as tile
from concourse import bacc
from concourse import mybir
from concourse.bass_utils import run_bass_kernel_spmd

N_CORES = 8
IMGS_PER_CORE = 2
C = 32
H = 256
W = 256
OH = 254
OW = 254
NS = 4          # row-splits per image
RS = 64 // NS   # r-rows per split chunk (16)
BR = 4 * RS // 4  # band rows per split = 16
NT = BR // 8    # 8-row groups per band per split (2)
F32 = mybir.dt.float32
BF16 = mybir.dt.bfloat16
LRELU = mybir.ActivationFunctionType.Lrelu


def build_nc(repeat=1):
    nc = bacc.Bacc()
    x_ext = nc.declare_dram_parameter(
        "xr", [2, IMGS_PER_CORE, 128, 64, W], BF16, isOutput=False
    )
    w_ext = nc.declare_dram_parameter("wr", [128, 2, 3, C], BF16, isOutput=False)
    b_ext = nc.declare_dram_parameter("biasr", [128], F32, isOutput=False)
    # y[img, split, j, m, hb, w] = out[m, 64*split + 16*j + hb, w]
    y_ext = nc.declare_dram_parameter(
        "y", [IMGS_PER_CORE, NS, 4, C, 16, OW], BF16, isOutput=True
    )

    with tile.TileContext(nc) as tc:
        with (
            tc.tile_pool(name="xp", bufs=2) as xpool,
            tc.tile_pool(name="const", bufs=1) as cpool,
            tc.tile_pool(name="ps", bufs=1, space="PSUM") as pspool,
            tc.tile_pool(name="outp", bufs=3) as opool,
        ):
            w_sb = cpool.tile([128, 2, 3, C], BF16)
            nc.sync.dma_start(out=w_sb, in_=w_ext[:])
            bias_half = cpool.tile([128, 1], F32)
            nc.sync.dma_start(out=bias_half, in_=b_ext[:].unsqueeze(1))

            bank_ctr = [0]
            for img_rep in range(IMGS_PER_CORE * repeat):
                img = img_rep % IMGS_PER_CORE
                for sp in range(NS):
                    xa = xpool.tile([128, RS, W], BF16, tag=f"A{sp}")
                    xb = xpool.tile([128, RS, W], BF16, tag=f"B{sp}")
                    r0 = RS * sp
                    nc.sync.dma_start(
                        out=xa, in_=x_ext[:][0][img][:, r0 : r0 + RS, :]
                    )
                    nc.sync.dma_start(
                        out=xb, in_=x_ext[:][1][img][:, r0 : r0 + RS, :]
                    )
                    ydst = y_ext[:][img][sp].rearrange("j m hb w -> (j m) hb w")
                    for t in range(NT):
                        stage = opool.tile([128, 8, OW], BF16)
                        last = sp == NS - 1 and t == NT - 1
                        if last:
                            # band-3 rows 254/255 are junk pad; define them
                            nc.vector.memset(stage[96:128, 6:8, :], 0.0)
                        for s in range(4):
                            v = s % 2
                            xt = xa if s < 2 else xb
                            plane = pspool.tile(
                                [128, 2, 256], F32, tag=f"bk{bank_ctr[0] % 8}"
                            )
                            bank_ctr[0] += 1
                            edge = last and s >= 2
                            for j in range(4):
                                r = 4 * j + 2 * t  # local r within chunk
                                if edge and j == 3:
                                    out_ap = plane[96:128, 0, 0:OW]
                                    for kw in range(3):
                                        nc.tensor.matmul(
                                            out_ap,
                                            w_sb[:, v, kw, :],
                                            xt[:, r, kw : kw + OW],
                                            start=(kw == 0),
                                            stop=(kw == 2),
                                            tile_position=(0, 96),
                                        )
                                else:
                                    for h2 in range(2):
                                        out_ap = plane[
                                            32 * j : 32 * (j + 1), h2, 0:OW
                                        ]
                                        for kw in range(3):
                                            nc.tensor.matmul(
                                                out_ap,
                                                w_sb[:, v, kw, :],
                                                xt[:, r + h2, kw : kw + OW],
                                                start=(kw == 0),
                                                stop=(kw == 2),
                                                tile_position=(0, 32 * j),
                                            )
                            if edge:
                                nc.scalar.activation(
                                    out=stage[0:96, s : s + 5 : 4, :],
                                    in_=plane[0:96, :, 0:OW],
                                    func=LRELU,
                                    bias=bias_half[0:96],
                                    scale=0.5,
                                    alpha=0.01,
                                )
                                nc.scalar.activation(
                                    out=stage[96:128, s, :],
                                    in_=plane[96:128, 0, 0:OW],
                                    func=LRELU,
                                    bias=bias_half[96:128],
                                    scale=0.5,
                                    alpha=0.01,
                                )
                            else:
                                nc.scalar.activation(
                                    out=stage[:, s : s + 5 : 4, :],
                                    in_=plane[:, :, 0:OW],
                                    func=LRELU,
                                    bias=bias_half,
                                    scale=0.5,
                                    alpha=0.01,
                                )
                        nc.sync.dma_start(
                            out=ydst[:, 8 * t : 8 * t + 8, :], in_=stage
                        )
    nc.compile()
    return nc


_CACHE = {}


def _get_nc(repeat=1):
    key = f"nc{repeat}"
    if key not in _CACHE:
        _CACHE[key] = build_nc(repeat)
    return _CACHE[key]


def make_wr_biasr(weight, bias):
    import ml_dtypes

    weight = np.asarray(weight, dtype=np.float32)
    bias = np.asarray(bias, dtype=np.float32)
    wr = np.zeros((128, 2, 3, C), dtype=np.float32)
    for p in range(128):
        s0, c0 = p // 32, p % 32
        if s0 < 3:
            wr[p, 0, :, :] = weight[:, c0, s0, :].T
        if 1 <= s0 <= 3:
            wr[p, 1, :, :] = weight[:, c0, s0 - 1, :].T
    wr = np.ascontiguousarray(wr.astype(ml_dtypes.bfloat16))
    biasr = np.ascontiguousarray(np.tile(bias * 0.5, 4).astype(np.float32))
    return wr, biasr


def make_xr(x):
    import ml_dtypes

    x = np.asarray(x, dtype=np.float32).astype(ml_dtypes.bfloat16)
    n = x.shape[0]
    x2 = np.concatenate(
        [x[:, :, 2:, :], np.zeros((n, C, 2, W), dtype=x.dtype)], axis=2
    )
    out = np.empty((2, n, 128, 64, W), dtype=x.dtype)
    for ci, src in enumerate((x, x2)):
        v = src.reshape(n, C, 64, 4, W).transpose(0, 3, 1, 2, 4)
        out[ci] = v.reshape(n, 128, 64, W)
    return np.ascontiguousarray(out)


def make_in_maps(x, weight, bias):
    xr = make_xr(x)
    wr, biasr = make_wr_biasr(weight, bias)
    return [
        {
            "xr": xr[:, IMGS_PER_CORE * i : IMGS_PER_CORE * (i + 1)],
            "wr": wr,
            "biasr": biasr,
        }
        for i in range(N_CORES)
    ]


def postprocess_y(y_banded):
    """[n, NS, 4, C, 16, OW] bf16 -> [n, C, 254, 254] f32."""
    y = np.asarray(y_banded).astype(np.float32)
    n = y.shape[0]
    # row = 64*split + 16*j + hb
    y = y.transpose(0, 3, 1, 2, 4, 5).reshape(n, C, 256, OW)
    return np.ascontiguousarray(y[:, :, 0:OH, :])


def kernel(x, weight, bias):
    in_maps = make_in_maps(x, weight, bias)
    nc = _get_nc()
    try:
        res = run_bass_kernel_spmd(nc, in_maps, core_ids=list(range(N_CORES)))
    except Exception:
        import time as _time

        _time.sleep(130)
        res = run_bass_kernel_spmd(nc, in_maps, core_ids=list(range(N_CORES)))
    y = np.concatenate(
        [postprocess_y(res.results[i]["y"]) for i in range(N_CORES)], axis=0
    )
    return y


# revision 10
# speedup vs baseline: 3.0728x; 1.0184x over previous
"""v4: like v3 but each image is processed in NS=4 row-splits with chunked
A/B loads, so compute starts after one small chunk DMA instead of the whole
image. Bands shrink to 16 output rows; y is declared [img, split, band, C,
16, OW]."""

import sys

if "/opt/trn_rl_repo" not in sys.path:
    sys.path.insert(0, "/opt/trn_rl_repo")

import numpy as np

import concourse.bass as bass
import concourse.tile as tile
from concourse import bacc
from concourse import mybir
from concourse.bass_utils import run_bass_kernel_spmd

N_CORES = 8
IMGS_PER_CORE = 2
C = 32
H = 256
W = 256
OH = 254
OW = 254
NS = 8          # row-splits per image
RS = 64 // NS   # r-rows per split chunk (16)
BR = 4 * RS // 4  # band rows per split = 16
NT = BR // 8    # 8-row groups per band per split (2)
F32 = mybir.dt.float32
BF16 = mybir.dt.bfloat16
LRELU = mybir.ActivationFunctionType.Lrelu


def build_nc(repeat=1):
    nc = bacc.Bacc()
    x_ext = nc.declare_dram_parameter(
        "xr", [2, IMGS_PER_CORE, 128, 64, W], BF16, isOutput=False
    )
    w_ext = nc.declare_dram_parameter("wr", [128, 2, 3, C], BF16, isOutput=False)
    b_ext = nc.declare_dram_parameter("biasr", [128], F32, isOutput=False)
    # y[img, split, j, m, hb, w] = out[m, 64*split + 16*j + hb, w]
    y_ext = nc.declare_dram_parameter(
        "y", [IMGS_PER_CORE, NS, 4, C, BR, OW], BF16, isOutput=True
    )

    with tile.TileContext(nc) as tc:
        with (
            tc.tile_pool(name="xp", bufs=2) as xpool,
            tc.tile_pool(name="const", bufs=1) as cpool,
            tc.tile_pool(name="ps", bufs=1, space="PSUM") as pspool,
            tc.tile_pool(name="outp", bufs=3) as opool,
        ):
            w_sb = cpool.tile([128, 2, 3, C], BF16)
            bias_half = cpool.tile([128, 1], F32)

            bank_ctr = [0]
            first_chunk = [True]
            for img_rep in range(IMGS_PER_CORE * repeat):
                img = img_rep % IMGS_PER_CORE
                for sp in range(NS):
                    xa = xpool.tile([128, RS, W], BF16, tag=f"A{sp}")
                    xb = xpool.tile([128, RS, W], BF16, tag=f"B{sp}")
                    r0 = RS * sp
                    nc.sync.dma_start(
                        out=xa, in_=x_ext[:][0][img][:, r0 : r0 + RS, :]
                    )
                    if first_chunk[0]:
                        first_chunk[0] = False
                        nc.sync.dma_start(out=w_sb, in_=w_ext[:])
                        nc.sync.dma_start(
                            out=bias_half, in_=b_ext[:].unsqueeze(1)
                        )
                    nc.sync.dma_start(
                        out=xb, in_=x_ext[:][1][img][:, r0 : r0 + RS, :]
                    )
                    ydst = y_ext[:][img][sp].rearrange("j m hb w -> (j m) hb w")
                    for t in range(NT):
                        stage = opool.tile([128, 8, OW], BF16)
                        last = sp == NS - 1 and t == NT - 1
                        if last:
                            # band-3 rows 254/255 are junk pad; define them
                            nc.vector.memset(stage[96:128, 6:8, :], 0.0)
                        for s in range(4):
                            v = s % 2
                            xt = xa if s < 2 else xb
                            plane = pspool.tile(
                                [128, 2, 256], F32, tag=f"bk{bank_ctr[0] % 8}"
                            )
                            bank_ctr[0] += 1
                            edge = last and s >= 2
                            for j in range(4):
                                r = (BR // 4) * j + 2 * t  # local r within chunk
                                if edge and j == 3:
                                    out_ap = plane[96:128, 0, 0:OW]
                                    for kw in range(3):
                                        nc.tensor.matmul(
                                            out_ap,
                                            w_sb[:, v, kw, :],
                                            xt[:, r, kw : kw + OW],
                                            start=(kw == 0),
                                            stop=(kw == 2),
                                            tile_position=(0, 96),
                                        )
                                else:
                                    for h2 in range(2):
                                        out_ap = plane[
                                            32 * j : 32 * (j + 1), h2, 0:OW
                                        ]
                                        for kw in range(3):
                                            nc.tensor.matmul(
                                                out_ap,
                                                w_sb[:, v, kw, :],
                                                xt[:, r + h2, kw : kw + OW],
                                                start=(kw == 0),
                                                stop=(kw == 2),
                                                tile_position=(0, 32 * j),
                                            )
                            if edge:
                                nc.scalar.activation(
                                    out=stage[0:96, s : s + 5 : 4, :],
                                    in_=plane[0:96, :, 0:OW],
                                    func=LRELU,
                                    bias=bias_half[0:96],
                                    scale=0.5,
                                    alpha=0.01,
                                )
                                nc.scalar.activation(
                                    out=stage[96:128, s, :],
                                    in_=plane[96:128, 0, 0:OW],
                                    func=LRELU,
                                    bias=bias_half[96:128],
                                    scale=0.5,
                                    alpha=0.01,
                                )
                            else:
                                nc.scalar.activation(
                                    out=stage[:, s : s + 5 : 4, :],
                                    in_=plane[:, :, 0:OW],
                                    func=LRELU,
                                    bias=bias_half,
                                    scale=0.5,
                                    alpha=0.01,
                                )
                            if img_rep == IMGS_PER_CORE * repeat - 1 and last:
                                # tail trim: store each round as soon as its
                                # eviction lands instead of one grouped DMA
                                ss = 8 * t + s
                                nc.sync.dma_start(
                                    out=ydst[:, ss : ss + 5 : 4, :],
                                    in_=stage[:, s : s + 5 : 4, :],
                                )
                        if img_rep == IMGS_PER_CORE * repeat - 1 and last:
                            pass  # per-round DMAs issued above
                        else:
                            nc.sync.dma_start(
                                out=ydst[:, 8 * t : 8 * t + 8, :], in_=stage
                            )
    nc.compile()
    return nc


_CACHE = {}


def _get_nc(repeat=1):
    key = f"nc{repeat}"
    if key not in _CACHE:
        _CACHE[key] = build_nc(repeat)
    return _CACHE[key]


def make_wr_biasr(weight, bias):
    import ml_dtypes

    weight = np.asarray(weight, dtype=np.float32)
    bias = np.asarray(bias, dtype=np.float32)
    wr = np.zeros((128, 2, 3, C), dtype=np.float32)
    for p in range(128):
        s0, c0 = p // 32, p % 32
        if s0 < 3:
            wr[p, 0, :, :] = weight[:, c0, s0, :].T
        if 1 <= s0 <= 3:
            wr[p, 1, :, :] = weight[:, c0, s0 - 1, :].T
    wr = np.ascontiguousarray(wr.astype(ml_dtypes.bfloat16))
    biasr = np.ascontiguousarray(np.tile(bias * 0.5, 4).astype(np.float32))
    return wr, biasr


def make_xr(x):
    import ml_dtypes

    x = np.asarray(x, dtype=np.float32).astype(ml_dtypes.bfloat16)
    n = x.shape[0]
    x2 = np.concatenate(
        [x[:, :, 2:, :], np.zeros((n, C, 2, W), dtype=x.dtype)], axis=2
    )
    out = np.empty((2, n, 128, 64, W), dtype=x.dtype)
    for ci, src in enumerate((x, x2)):
        v = src.reshape(n, C, 64, 4, W).transpose(0, 3, 1, 2, 4)
        out[ci] = v.reshape(n, 128, 64, W)
    return np.ascontiguousarray(out)


def make_in_maps(x, weight, bias):
    xr = make_xr(x)
    wr, biasr = make_wr_biasr(weight, bias)
    return [
        {
            "xr": xr[:, IMGS_PER_CORE * i : IMGS_PER_CORE * (i + 1)],
            "wr": wr,
            "biasr": biasr,
        }
        for i in range(N_CORES)
    ]


def postprocess_y(y_banded):
    """[n, NS, 4, C, 16, OW] bf16 -> [n, C, 254, 254] f32."""
    y = np.asarray(y_banded).astype(np.float32)
    n = y.shape[0]
    # row = 64*split + 16*j + hb
    y = y.transpose(0, 3, 1, 2, 4, 5).reshape(n, C, 256, OW)
    return np.ascontiguousarray(y[:, :, 0:OH, :])


def kernel(x, weight, bias):
    in_maps = make_in_maps(x, weight, bias)
    nc = _get_nc()
    try:
        res = run_bass_kernel_spmd(nc, in_maps, core_ids=list(range(N_CORES)))
    except Exception:
        import time as _time

        _time.sleep(130)
        res = run_bass_kernel_spmd(nc, in_maps, core_ids=list(range(N_CORES)))
    y = np.concatenate(
        [postprocess_y(res.results[i]["y"]) for i in range(N_CORES)], axis=0
    )
    return y


# revision 12
# speedup vs baseline: 4.0675x; 1.3237x over previous
"""v7: d-stacked matmuls. One K=128/M=128/N=254 matmul computes one conv tap
for FOUR consecutive output rows: contraction partitions are (d, c_in) with a
block-diagonal weight (w if d==d' else 0), output partitions are (d, c_out).
Three row-phase copies of x (phi = kh) make every tap's rhs a rectangular AP:
  copy_phi[32g+c, rr, w] = x[c, 4rr+g+phi, w]   (zero-padded past row 255)
so tap (kh,kw) of output group q (rows 4q..4q+3) reads
  rhs = copy_kh[:, rr=q, kw:kw+254].
9 taps accumulate into one PSUM plane slot. 2.25 matmul-rows per output
pixel -- the cost-model floor for K,M<=128.

y is declared row-interleaved [img, d, C, 64, OW] (y[img,d,m,q,w] = out row
4q+d; rows 254/255 junk from zero-pad); host reassembles.
"""

import sys

if "/opt/trn_rl_repo" not in sys.path:
    sys.path.insert(0, "/opt/trn_rl_repo")

import numpy as np

import concourse.bass as bass
import concourse.tile as tile
from concourse import bacc
from concourse import mybir
from concourse.bass_utils import run_bass_kernel_spmd

N_CORES = 8
IMGS_PER_CORE = 2
C = 32
H = 256
W = 256
OH = 254
OW = 254
NS = 8           # group-chunks per image
QS = 64 // NS    # groups per chunk (8) = 32 output rows
F32 = mybir.dt.float32
BF16 = mybir.dt.bfloat16
LRELU = mybir.ActivationFunctionType.Lrelu


def build_nc(repeat=1):
    nc = bacc.Bacc()
    x_ext = nc.declare_dram_parameter(
        "xr", [3, IMGS_PER_CORE, 128, 64, W], BF16, isOutput=False
    )
    # w9[32d+c, tap, 32d'+m] = weight[m, c, kh, kw] if d==d' else 0
    w_ext = nc.declare_dram_parameter("w9", [128, 9, 128], BF16, isOutput=False)
    b_ext = nc.declare_dram_parameter("biasr", [128], F32, isOutput=False)
    # y[img, d, m, q, w] = out[m, 4q+d, w]; rows 254/255 are junk pad
    y_ext = nc.declare_dram_parameter(
        "y", [IMGS_PER_CORE, 4, C, 64, OW], BF16, isOutput=True
    )

    with tile.TileContext(nc) as tc:
        with (
            tc.tile_pool(name="xp", bufs=2) as xpool,
            tc.tile_pool(name="const", bufs=1) as cpool,
            tc.tile_pool(name="ps", bufs=1, space="PSUM") as pspool,
            tc.tile_pool(name="outp", bufs=3) as opool,
        ):
            w_sb = cpool.tile([128, 9, 128], BF16)
            bias_half = cpool.tile([128, 1], F32)
            nc.sync.dma_start(out=w_sb, in_=w_ext[:])
            nc.sync.dma_start(out=bias_half, in_=b_ext[:].unsqueeze(1))

            # PE p-state warmup: dead N=8 matmuls during the initial DMA
            # wait keep the PE busy so real matmuls start at 2.4 GHz.
            warm = cpool.tile([128, 16], BF16)
            nc.vector.memset(warm, 0.0)
            wplane = pspool.tile([128, 2, 256], F32, tag="bk7")
            for _ in range(650):
                nc.tensor.matmul(
                    wplane[0:16, 0, 0:8],
                    warm[:, 0:16],
                    warm[:, 0:8],
                    start=True,
                    stop=True,
                )

            bank_ctr = [0]
            for img_rep in range(IMGS_PER_CORE * repeat):
                img = img_rep % IMGS_PER_CORE
                for sp in range(NS):
                    q0 = QS * sp
                    xc = []
                    for phi in range(3):
                        xt = xpool.tile([128, QS, W], BF16, tag=f"c{phi}")
                        xc.append(xt)
                        nc.sync.dma_start(
                            out=xt,
                            in_=x_ext[:][phi][img][:, q0 : q0 + QS, :],
                        )
                    ydst = y_ext[:][img].rearrange("d m q w -> (d m) q w")
                    stage = opool.tile([128, QS, OW], BF16)
                    tail = img_rep == IMGS_PER_CORE * repeat - 1 and sp == NS - 1
                    ng = 1 if tail else 2  # groups per plane (tail: finer)
                    for u in range(QS // ng):
                        plane = pspool.tile(
                            [128, 2, 256], F32, tag=f"bk{bank_ctr[0] % 8}"
                        )
                        bank_ctr[0] += 1
                        for qq in range(ng):
                            for kh in range(3):
                                for kw in range(3):
                                    tap = 3 * kh + kw
                                    nc.tensor.matmul(
                                        plane[:, qq, 0:OW],
                                        w_sb[:, tap, :],
                                        xc[kh][:, ng * u + qq, kw : kw + OW],
                                        start=(tap == 0),
                                        stop=(tap == 8),
                                    )
                        nc.scalar.activation(
                            out=stage[:, ng * u : ng * u + ng, :],
                            in_=plane[:, 0:ng, 0:OW],
                            func=LRELU,
                            bias=bias_half,
                            scale=0.5,
                            alpha=0.01,
                        )
                        if tail:
                            # store per plane so the tail shrinks
                            nc.sync.dma_start(
                                out=ydst[:, q0 + ng * u : q0 + ng * u + ng, :],
                                in_=stage[:, ng * u : ng * u + ng, :],
                            )
                    if not tail:
                        nc.sync.dma_start(
                            out=ydst[:, q0 : q0 + QS, :], in_=stage
                        )
    nc.compile()
    return nc


_CACHE = {}


def _get_nc(repeat=1):
    key = f"nc{repeat}"
    if key not in _CACHE:
        _CACHE[key] = build_nc(repeat)
    return _CACHE[key]


def make_w9_biasr(weight, bias):
    import ml_dtypes

    weight = np.asarray(weight, dtype=np.float32)
    bias = np.asarray(bias, dtype=np.float32)
    w9 = np.zeros((128, 9, 128), dtype=np.float32)
    for d in range(4):
        for kh in range(3):
            for kw in range(3):
                # w9[32d+c, 3kh+kw, 32d+m] = weight[m, c, kh, kw]
                w9[32 * d : 32 * d + 32, 3 * kh + kw, 32 * d : 32 * d + 32] = (
                    weight[:, :, kh, kw].T
                )
    w9 = np.ascontiguousarray(w9.astype(ml_dtypes.bfloat16))
    biasr = np.ascontiguousarray(np.tile(bias * 0.5, 4).astype(np.float32))
    return w9, biasr


def make_xr(x):
    """xr[phi, n, 32g+c, rr, w] = x[n, c, 4rr+g+phi, w], zero past row 255."""
    import ml_dtypes

    x = np.asarray(x, dtype=np.float32).astype(ml_dtypes.bfloat16)
    n = x.shape[0]
    out = np.empty((3, n, 128, 64, W), dtype=x.dtype)
    for phi in range(3):
        if phi:
            src = np.concatenate(
                [x[:, :, phi:, :], np.zeros((n, C, phi, W), dtype=x.dtype)],
                axis=2,
            )
        else:
            src = x
        v = src.reshape(n, C, 64, 4, W).transpose(0, 3, 1, 2, 4)
        out[phi] = v.reshape(n, 128, 64, W)
    return np.ascontiguousarray(out)


def make_in_maps(x, weight, bias):
    xr = make_xr(x)
    w9, biasr = make_w9_biasr(weight, bias)
    return [
        {
            "xr": xr[:, IMGS_PER_CORE * i : IMGS_PER_CORE * (i + 1)],
            "w9": w9,
            "biasr": biasr,
        }
        for i in range(N_CORES)
    ]


def postprocess_y(y_banded):
    """[n, 4, C, 64, OW] bf16 (row = 4q+d) -> [n, C, 254, 254] f32."""
    y = np.asarray(y_banded).astype(np.float32)
    n = y.shape[0]
    # [n, d, m, q, w] -> [n, m, q, d, w] -> rows 4q+d
    y = y.transpose(0, 2, 3, 1, 4).reshape(n, C, 256, OW)
    return np.ascontiguousarray(y[:, :, 0:OH, :])


def kernel(x, weight, bias):
    in_maps = make_in_maps(x, weight, bias)
    nc = _get_nc()
    try:
        res = run_bass_kernel_spmd(nc, in_maps, core_ids=list(range(N_CORES)))
    except Exception:
        # transient device fault (axon terminal resets itself in ~2 min)
        import time as _time

        _time.sleep(130)
        res = run_bass_kernel_spmd(nc, in_maps, core_ids=list(range(N_CORES)))
    y = np.concatenate(
        [postprocess_y(res.results[i]["y"]) for i in range(N_CORES)], axis=0
    )
    return y


# revision 13
# speedup vs baseline: 4.0760x; 1.0021x over previous
"""v7: d-stacked matmuls. One K=128/M=128/N=254 matmul computes one conv tap
for FOUR consecutive output rows: contraction partitions are (d, c_in) with a
block-diagonal weight (w if d==d' else 0), output partitions are (d, c_out).
Three row-phase copies of x (phi = kh) make every tap's rhs a rectangular AP:
  copy_phi[32g+c, rr, w] = x[c, 4rr+g+phi, w]   (zero-padded past row 255)
so tap (kh,kw) of output group q (rows 4q..4q+3) reads
  rhs = copy_kh[:, rr=q, kw:kw+254].
9 taps accumulate into one PSUM plane slot. 2.25 matmul-rows per output
pixel -- the cost-model floor for K,M<=128.

y is declared row-interleaved [img, d, C, 64, OW] (y[img,d,m,q,w] = out row
4q+d; rows 254/255 junk from zero-pad); host reassembles.
"""

import sys

if "/opt/trn_rl_repo" not in sys.path:
    sys.path.insert(0, "/opt/trn_rl_repo")

import numpy as np

import concourse.bass as bass
import concourse.tile as tile
from concourse import bacc
from concourse import mybir
from concourse.bass_utils import run_bass_kernel_spmd

N_CORES = 8
IMGS_PER_CORE = 2
C = 32
H = 256
W = 256
OH = 254
OW = 254
NS = 8           # group-chunks per image
QS = 64 // NS    # groups per chunk (8) = 32 output rows
F32 = mybir.dt.float32
BF16 = mybir.dt.bfloat16
LRELU = mybir.ActivationFunctionType.Lrelu


def build_nc(repeat=1):
    nc = bacc.Bacc()
    x_ext = nc.declare_dram_parameter(
        "xr", [3, IMGS_PER_CORE, 128, 64, W], BF16, isOutput=False
    )
    # w9[32d+c, tap, 32d'+m] = weight[m, c, kh, kw] if d==d' else 0
    w_ext = nc.declare_dram_parameter("w9", [128, 9, 128], BF16, isOutput=False)
    b_ext = nc.declare_dram_parameter("biasr", [128], F32, isOutput=False)
    # y[img, d, m, q, w] = out[m, 4q+d, w]; rows 254/255 are junk pad
    y_ext = nc.declare_dram_parameter(
        "y", [IMGS_PER_CORE, 4, C, 64, OW], BF16, isOutput=True
    )

    with tile.TileContext(nc) as tc:
        with (
            tc.tile_pool(name="xp", bufs=2) as xpool,
            tc.tile_pool(name="const", bufs=1) as cpool,
            tc.tile_pool(name="ps", bufs=1, space="PSUM") as pspool,
            tc.tile_pool(name="outp", bufs=3) as opool,
        ):
            w_sb = cpool.tile([128, 9, 128], BF16)
            bias_half = cpool.tile([128, 1], F32)
            nc.sync.dma_start(out=w_sb, in_=w_ext[:])
            nc.sync.dma_start(out=bias_half, in_=b_ext[:].unsqueeze(1))

            # PE p-state warmup: dead N=8 matmuls during the initial DMA
            # wait keep the PE busy so real matmuls start at 2.4 GHz.
            warm = cpool.tile([128, 16], BF16)
            nc.vector.memset(warm, 0.0)
            wplane = pspool.tile([128, 2, 256], F32, tag="bk7")
            for _ in range(560):
                nc.tensor.matmul(
                    wplane[0:16, 0, 0:8],
                    warm[:, 0:16],
                    warm[:, 0:8],
                    start=True,
                    stop=True,
                )

            bank_ctr = [0]
            for img_rep in range(IMGS_PER_CORE * repeat):
                img = img_rep % IMGS_PER_CORE
                for sp in range(NS):
                    q0 = QS * sp
                    xc = []
                    for phi in range(3):
                        xt = xpool.tile([128, QS, W], BF16, tag=f"c{phi}")
                        xc.append(xt)
                        if img_rep == 0 and sp == 0:
                            # split so the first plane's rows land sooner
                            nc.sync.dma_start(
                                out=xt[:, 0:2, :],
                                in_=x_ext[:][phi][img][:, q0 : q0 + 2, :],
                            )
                            nc.sync.dma_start(
                                out=xt[:, 2:QS, :],
                                in_=x_ext[:][phi][img][:, q0 + 2 : q0 + QS, :],
                            )
                        else:
                            nc.sync.dma_start(
                                out=xt,
                                in_=x_ext[:][phi][img][:, q0 : q0 + QS, :],
                            )
                    ydst = y_ext[:][img].rearrange("d m q w -> (d m) q w")
                    stage = opool.tile([128, QS, OW], BF16)
                    tail = img_rep == IMGS_PER_CORE * repeat - 1 and sp == NS - 1
                    ng = 1 if tail else 2  # groups per plane (tail: finer)
                    for u in range(QS // ng):
                        plane = pspool.tile(
                            [128, 2, 256], F32, tag=f"bk{bank_ctr[0] % 8}"
                        )
                        bank_ctr[0] += 1
                        for qq in range(ng):
                            for kh in range(3):
                                for kw in range(3):
                                    tap = 3 * kh + kw
                                    nc.tensor.matmul(
                                        plane[:, qq, 0:OW],
                                        w_sb[:, tap, :],
                                        xc[kh][:, ng * u + qq, kw : kw + OW],
                                        start=(tap == 0),
                                        stop=(tap == 8),
                                    )
                        nc.scalar.activation(
                            out=stage[:, ng * u : ng * u + ng, :],
                            in_=plane[:, 0:ng, 0:OW],
                            func=LRELU,
                            bias=bias_half,
                            scale=0.5,
                            alpha=0.01,
                        )
                        if tail:
                            # store per plane so the tail shrinks
                            nc.sync.dma_start(
                                out=ydst[:, q0 + ng * u : q0 + ng * u + ng, :],
                                in_=stage[:, ng * u : ng * u + ng, :],
                            )
                    if not tail:
                        nc.sync.dma_start(
                            out=ydst[:, q0 : q0 + QS, :], in_=stage
                        )
    nc.compile()
    return nc


_CACHE = {}


def _get_nc(repeat=1):
    key = f"nc{repeat}"
    if key not in _CACHE:
        _CACHE[key] = build_nc(repeat)
    return _CACHE[key]


def make_w9_biasr(weight, bias):
    import ml_dtypes

    weight = np.asarray(weight, dtype=np.float32)
    bias = np.asarray(bias, dtype=np.float32)
    w9 = np.zeros((128, 9, 128), dtype=np.float32)
    for d in range(4):
        for kh in range(3):
            for kw in range(3):
                # w9[32d+c, 3kh+kw, 32d+m] = weight[m, c, kh, kw]
                w9[32 * d : 32 * d + 32, 3 * kh + kw, 32 * d : 32 * d + 32] = (
                    weight[:, :, kh, kw].T
                )
    w9 = np.ascontiguousarray(w9.astype(ml_dtypes.bfloat16))
    biasr = np.ascontiguousarray(np.tile(bias * 0.5, 4).astype(np.float32))
    return w9, biasr


def make_xr(x):
    """xr[phi, n, 32g+c, rr, w] = x[n, c, 4rr+g+phi, w], zero past row 255."""
    import ml_dtypes

    x = np.asarray(x, dtype=np.float32).astype(ml_dtypes.bfloat16)
    n = x.shape[0]
    out = np.empty((3, n, 128, 64, W), dtype=x.dtype)
    for phi in range(3):
        if phi:
            src = np.concatenate(
                [x[:, :, phi:, :], np.zeros((n, C, phi, W), dtype=x.dtype)],
                axis=2,
            )
        else:
            src = x
        v = src.reshape(n, C, 64, 4, W).transpose(0, 3, 1, 2, 4)
        out[phi] = v.reshape(n, 128, 64, W)
    return np.ascontiguousarray(out)


def make_in_maps(x, weight, bias):
    xr = make_xr(x)
    w9, biasr = make_w9_biasr(weight, bias)
    return [
        {
            "xr": xr[:, IMGS_PER_CORE * i : IMGS_PER_CORE * (i + 1)],
            "w9": w9,
            "biasr": biasr,
        }
        for i in range(N_CORES)
    ]


def postprocess_y(y_banded):
    """[n, 4, C, 64, OW] bf16 (row = 4q+d) -> [n, C, 254, 254] f32."""
    y = np.asarray(y_banded).astype(np.float32)
    n = y.shape[0]
    # [n, d, m, q, w] -> [n, m, q, d, w] -> rows 4q+d
    y = y.transpose(0, 2, 3, 1, 4).reshape(n, C, 256, OW)
    return np.ascontiguousarray(y[:, :, 0:OH, :])


def kernel(x, weight, bias):
    in_maps = make_in_maps(x, weight, bias)
    nc = _get_nc()
    try:
        res = run_bass_kernel_spmd(nc, in_maps, core_ids=list(range(N_CORES)))
    except Exception:
        # transient device fault (axon terminal resets itself in ~2 min)
        import time as _time

        _time.sleep(130)
        res = run_bass_kernel_spmd(nc, in_maps, core_ids=list(range(N_CORES)))
    y = np.concatenate(
        [postprocess_y(res.results[i]["y"]) for i in range(N_CORES)], axis=0
    )
    return y


# revision 19
# speedup vs baseline: 6.5984x; 1.6188x over previous
"""v11: cross-group offset matmuls -- 6 taps per 4-row group, ONE x copy.

Single row-phase copy: xr[img, 32b+c, rr, w] = x[c, 4rr+b, w] (rr=64 zeros).
Reading free row q gives partition block b the image row 4q+b (rho=b);
reading free row q+1 gives rho=4+b. Weight for (offset off, shift delta):
  w6[32b+c, 3*off+delta, 32d+m] = w[m, c, rho-d, delta],  rho = 4*off+b,
  nonzero wherever 0 <= rho-d <= 2.
One matmul then serves ALL (d, kh) pairs with d+kh = rho at once; the six
(off, delta) matmuls cover all 36 (d, kh, delta) terms exactly once:
  out[(d,m), n] = sum_{kh,kw} w[m,c,kh,kw] x[c, 4q+d+kh, n+kw].
6 matmuls x N=254 per 4 output rows (1.5 rows/pixel) vs 9 for the
block-diagonal form. PE ~81 us, input DMA ~28 us (one copy).

y is row-interleaved [img, d, C, 64, OW] (row = 4q+d; rows 254/255 junk);
host reassembles.
"""

import sys

if "/opt/trn_rl_repo" not in sys.path:
    sys.path.insert(0, "/opt/trn_rl_repo")

import numpy as np

import concourse.bass as bass
import concourse.tile as tile
from concourse import bacc
from concourse import mybir
from concourse.bass_utils import run_bass_kernel_spmd

N_CORES = 8
IMGS_PER_CORE = 2
C = 32
H = 256
W = 256
OH = 254
OW = 254
NS = 8           # group-chunks per image
QS = 64 // NS    # groups per chunk (8) = 32 output rows
F32 = mybir.dt.float32
BF16 = mybir.dt.bfloat16
LRELU = mybir.ActivationFunctionType.Lrelu


def build_nc(repeat=1):
    nc = bacc.Bacc()
    x_ext = nc.declare_dram_parameter(
        "xr", [IMGS_PER_CORE, 128, 65, W], BF16, isOutput=False
    )
    x2_ext = nc.declare_dram_parameter(
        "xc2", [IMGS_PER_CORE, 128, 64, W], BF16, isOutput=False
    )
    w_ext = nc.declare_dram_parameter("w6", [128, 7, 128], BF16, isOutput=False)
    b_ext = nc.declare_dram_parameter("biasr", [128], F32, isOutput=False)
    # y[img, d, m, q, w] = out[m, 4q+d, w]; rows 254/255 are junk pad
    y_ext = nc.declare_dram_parameter(
        "y", [IMGS_PER_CORE, 4, C, 64, OW], BF16, isOutput=True
    )

    with tile.TileContext(nc) as tc:
        with (
            tc.tile_pool(name="xp", bufs=2) as xpool,
            tc.tile_pool(name="const", bufs=1) as cpool,
            tc.tile_pool(name="ps", bufs=1, space="PSUM") as pspool,
            tc.tile_pool(name="outp", bufs=3) as opool,
        ):
            w_sb = cpool.tile([128, 7, 128], BF16)
            bias_half = cpool.tile([128, 1], F32)
            nc.sync.dma_start(out=w_sb, in_=w_ext[:])
            nc.sync.dma_start(out=bias_half, in_=b_ext[:].unsqueeze(1))

            # PE p-state warmup: dead N=8 matmuls during the initial DMA
            # wait keep the PE busy so real matmuls start at 2.4 GHz.
            warm = cpool.tile([128, 16], BF16)
            nc.gpsimd.memset(warm, 0.0)
            wplane = pspool.tile([128, 2, 256], F32, tag="bk7")
            for _ in range(530):
                nc.tensor.matmul(
                    wplane[0:16, 0, 0:8],
                    warm[:, 0:16],
                    warm[:, 0:8],
                    start=True,
                    stop=True,
                )

            bank_ctr = [0]

            def load_chunk(img_rep, sp):
                """Emit chunk DMAs; last chunk of an image loads the QS+1
                zero row, others load QS rows (the q+1 read of the last
                group crosses into the NEXT chunk's row 0)."""
                img = img_rep % IMGS_PER_CORE
                q0 = QS * sp
                five = (img_rep * NS + sp) % 4 != 0
                lastsp = sp == NS - 1
                rows = QS + 1 if lastsp else QS
                xt = xpool.tile([128, QS + 1, W], BF16, tag="x")
                xt2 = None
                if five:
                    xt2 = xpool.tile([128, QS, W], BF16, tag="x2")
                    nc.sync.dma_start(
                        out=xt2, in_=x2_ext[:][img][:, q0 : q0 + QS, :]
                    )
                if img_rep == 0 and sp == 0:
                    nc.sync.dma_start(
                        out=xt[:, 0:3, :],
                        in_=x_ext[:][img][:, q0 : q0 + 3, :],
                    )
                    nc.sync.dma_start(
                        out=xt[:, 3:rows, :],
                        in_=x_ext[:][img][:, q0 + 3 : q0 + rows, :],
                    )
                else:
                    nc.sync.dma_start(
                        out=xt[:, 0:rows, :],
                        in_=x_ext[:][img][:, q0 : q0 + rows, :],
                    )
                return xt, xt2, five

            nreps = IMGS_PER_CORE * repeat
            cur = load_chunk(0, 0)
            for img_rep in range(nreps):
                img = img_rep % IMGS_PER_CORE
                for sp in range(NS):
                    q0 = QS * sp
                    xt, xt2, five = cur
                    # prefetch next chunk BEFORE this chunk's out-DMA so the
                    # cross-chunk q+1 read can't deadlock behind it
                    if sp < NS - 1:
                        nxt = load_chunk(img_rep, sp + 1)
                    elif img_rep < nreps - 1:
                        nxt = load_chunk(img_rep + 1, 0)
                    else:
                        nxt = None
                    xt_nx = nxt[0] if (nxt is not None and sp < NS - 1) else xt
                    ydst = y_ext[:][img].rearrange("d m q w -> (d m) q w")
                    stage = opool.tile([128, QS, OW], BF16)
                    tail = img_rep == IMGS_PER_CORE * repeat - 1 and sp == NS - 1
                    ng = 1 if tail else 2  # groups per plane (tail: finer)
                    for u in range(QS // ng):
                        plane = pspool.tile(
                            [128, 2, 256], F32, tag=f"bk{bank_ctr[0] % 8}"
                        )
                        bank_ctr[0] += 1
                        for qq in range(ng):
                            ql = ng * u + qq  # local group index in chunk
                            def rhs(offrow, delta):
                                if offrow == QS and sp < NS - 1:
                                    return xt_nx[:, 0, delta : delta + OW]
                                return xt[:, offrow, delta : delta + OW]

                            if five:
                                # taps 0-2: A deltas @q; tap 3: off1 d0 @q+1;
                                # tap 6: copy2 covers (rho>=4, delta in {1,2})
                                for tap in range(4):
                                    off = tap // 3
                                    delta = tap % 3
                                    nc.tensor.matmul(
                                        plane[:, qq, 0:OW],
                                        w_sb[:, tap, :],
                                        rhs(ql + off, delta),
                                        start=(tap == 0),
                                        stop=False,
                                    )
                                nc.tensor.matmul(
                                    plane[:, qq, 0:OW],
                                    w_sb[:, 6, :],
                                    xt2[:, ql, 0:OW],
                                    start=False,
                                    stop=True,
                                )
                            else:
                                for off in range(2):
                                    for delta in range(3):
                                        tap = 3 * off + delta
                                        nc.tensor.matmul(
                                            plane[:, qq, 0:OW],
                                            w_sb[:, tap, :],
                                            rhs(ql + off, delta),
                                            start=(tap == 0),
                                            stop=(tap == 5),
                                        )
                        nc.scalar.activation(
                            out=stage[:, ng * u : ng * u + ng, :],
                            in_=plane[:, 0:ng, 0:OW],
                            func=LRELU,
                            bias=bias_half,
                            scale=0.5,
                            alpha=0.01,
                        )
                        if tail:
                            nc.sync.dma_start(
                                out=ydst[:, q0 + ng * u : q0 + ng * u + ng, :],
                                in_=stage[:, ng * u : ng * u + ng, :],
                            )
                    if not tail:
                        nc.sync.dma_start(
                            out=ydst[:, q0 : q0 + QS, :], in_=stage
                        )
                    cur = nxt
    nc.compile()
    return nc


_CACHE = {}


def _get_nc(repeat=1):
    key = f"nc{repeat}"
    if key not in _CACHE:
        _CACHE[key] = build_nc(repeat)
    return _CACHE[key]


def make_w6_biasr(weight, bias):
    import ml_dtypes

    weight = np.asarray(weight, dtype=np.float32)
    bias = np.asarray(bias, dtype=np.float32)
    w6 = np.zeros((128, 7, 128), dtype=np.float32)
    for off in range(2):
        for delta in range(3):
            tap = 3 * off + delta
            for b in range(4):
                rho = 4 * off + b
                for d in range(4):
                    kh = rho - d
                    if 0 <= kh <= 2:
                        w6[
                            32 * b : 32 * b + 32,
                            tap,
                            32 * d : 32 * d + 32,
                        ] = weight[:, :, kh, delta].T
    # tap 6 = copy2 slots (rho, delta) = [(4,1),(4,2),(5,1),(5,2)]
    for s, (rho, delta) in enumerate([(4, 1), (4, 2), (5, 1), (5, 2)]):
        for d in range(4):
            kh = rho - d
            if 0 <= kh <= 2:
                w6[32 * s : 32 * s + 32, 6, 32 * d : 32 * d + 32] = (
                    weight[:, :, kh, delta].T
                )
    w6 = np.ascontiguousarray(w6.astype(ml_dtypes.bfloat16))
    biasr = np.ascontiguousarray(np.tile(bias * 0.5, 4).astype(np.float32))
    return w6, biasr


def make_xr(x):
    """xr[n, 32b+c, rr, w] = x[n, c, 4rr+b, w]; rr=64 is zeros."""
    import ml_dtypes

    x = np.asarray(x, dtype=np.float32).astype(ml_dtypes.bfloat16)
    n = x.shape[0]
    v = x.reshape(n, C, 64, 4, W).transpose(0, 3, 1, 2, 4).reshape(n, 128, 64, W)
    out = np.zeros((n, 128, 65, W), dtype=x.dtype)
    out[:, :, 0:64, :] = v
    return np.ascontiguousarray(out)


def make_xc2(x):
    """xc2[n, 32s+c, rr, w] = x[n, c, 4rr+4+s//2, w + 1+s%2] (zero-pad OOB)."""
    import ml_dtypes

    x = np.asarray(x, dtype=np.float32).astype(ml_dtypes.bfloat16)
    n = x.shape[0]
    out = np.zeros((n, 128, 64, W), dtype=x.dtype)
    for s, (rho, delta) in enumerate([(4, 1), (4, 2), (5, 1), (5, 2)]):
        rows = np.zeros((n, C, 64, W), dtype=x.dtype)
        hs = np.arange(64) * 4 + rho
        valid = hs < H
        src_rows = x[:, :, hs[valid], :]
        rows[:, :, valid, : W - delta] = src_rows[:, :, :, delta:]
        out[:, 32 * s : 32 * s + 32, :, :] = rows
    return np.ascontiguousarray(out)


def make_in_maps(x, weight, bias):
    xr = make_xr(x)
    xc2 = make_xc2(x)
    w6, biasr = make_w6_biasr(weight, bias)
    return [
        {
            "xr": xr[IMGS_PER_CORE * i : IMGS_PER_CORE * (i + 1)],
            "xc2": xc2[IMGS_PER_CORE * i : IMGS_PER_CORE * (i + 1)],
            "w6": w6,
            "biasr": biasr,
        }
        for i in range(N_CORES)
    ]


def postprocess_y(y_banded):
    """[n, 4, C, 64, OW] bf16 (row = 4q+d) -> [n, C, 254, 254] f32."""
    y = np.asarray(y_banded).astype(np.float32)
    n = y.shape[0]
    y = y.transpose(0, 2, 3, 1, 4).reshape(n, C, 256, OW)
    return np.ascontiguousarray(y[:, :, 0:OH, :])


def kernel(x, weight, bias):
    in_maps = make_in_maps(x, weight, bias)
    nc = _get_nc()
    res = None
    for attempt in range(4):
        try:
            res = run_bass_kernel_spmd(
                nc, in_maps, core_ids=list(range(N_CORES))
            )
            break
        except Exception:
            # transient device fault (axon terminal resets itself in ~2 min);
            # a wedge can persist through one reset window, so retry a few
            # times with full waits
            if attempt == 3:
                raise
            import time as _time

            _time.sleep(150)
    y = np.concatenate(
        [postprocess_y(res.results[i]["y"]) for i in range(N_CORES)], axis=0
    )
    return y


# revision 20
# speedup vs baseline: 6.7256x; 1.0193x over previous
"""v11: cross-group offset matmuls -- 6 taps per 4-row group, ONE x copy.

Single row-phase copy: xr[img, 32b+c, rr, w] = x[c, 4rr+b, w] (rr=64 zeros).
Reading free row q gives partition block b the image row 4q+b (rho=b);
reading free row q+1 gives rho=4+b. Weight for (offset off, shift delta):
  w6[32b+c, 3*off+delta, 32d+m] = w[m, c, rho-d, delta],  rho = 4*off+b,
  nonzero wherever 0 <= rho-d <= 2.
One matmul then serves ALL (d, kh) pairs with d+kh = rho at once; the six
(off, delta) matmuls cover all 36 (d, kh, delta) terms exactly once:
  out[(d,m), n] = sum_{kh,kw} w[m,c,kh,kw] x[c, 4q+d+kh, n+kw].
6 matmuls x N=254 per 4 output rows (1.5 rows/pixel) vs 9 for the
block-diagonal form. PE ~81 us, input DMA ~28 us (one copy).

y is row-interleaved [img, d, C, 64, OW] (row = 4q+d; rows 254/255 junk);
host reassembles.
"""

import sys

if "/opt/trn_rl_repo" not in sys.path:
    sys.path.insert(0, "/opt/trn_rl_repo")

import numpy as np

import concourse.bass as bass
import concourse.tile as tile
from concourse import bacc
from concourse import mybir
from concourse.bass_utils import run_bass_kernel_spmd

N_CORES = 8
IMGS_PER_CORE = 2
C = 32
H = 256
W = 256
OH = 254
OW = 254
NS = 8           # group-chunks per image
QS = 64 // NS    # groups per chunk (8) = 32 output rows
F32 = mybir.dt.float32
BF16 = mybir.dt.bfloat16
LRELU = mybir.ActivationFunctionType.Lrelu


def build_nc(repeat=1):
    nc = bacc.Bacc()
    x_ext = nc.declare_dram_parameter(
        "xr", [IMGS_PER_CORE, 128, 65, W], BF16, isOutput=False
    )
    x2_ext = nc.declare_dram_parameter(
        "xc2", [IMGS_PER_CORE, 128, 64, W], BF16, isOutput=False
    )
    w_ext = nc.declare_dram_parameter("w6", [128, 7, 128], BF16, isOutput=False)
    b_ext = nc.declare_dram_parameter("biasr", [128], F32, isOutput=False)
    # y[img, d, m, q, w] = out[m, 4q+d, w]; rows 254/255 are junk pad
    y_ext = nc.declare_dram_parameter(
        "y", [IMGS_PER_CORE, 4, C, 64, OW], BF16, isOutput=True
    )

    with tile.TileContext(nc) as tc:
        with (
            tc.tile_pool(name="xp", bufs=3) as xpool,
            tc.tile_pool(name="const", bufs=1) as cpool,
            tc.tile_pool(name="ps", bufs=1, space="PSUM") as pspool,
            tc.tile_pool(name="outp", bufs=3) as opool,
        ):
            w_sb = cpool.tile([128, 7, 128], BF16)
            bias_half = cpool.tile([128, 1], F32)
            nc.sync.dma_start(out=w_sb, in_=w_ext[:])
            nc.sync.dma_start(out=bias_half, in_=b_ext[:].unsqueeze(1))

            # PE p-state warmup: dead N=8 matmuls during the initial DMA
            # wait keep the PE busy so real matmuls start at 2.4 GHz.
            warm = cpool.tile([128, 16], BF16)
            nc.gpsimd.memset(warm, 0.0)
            wplane = pspool.tile([128, 2, 256], F32, tag="bk7")
            for _ in range(530):
                nc.tensor.matmul(
                    wplane[0:16, 0, 0:8],
                    warm[:, 0:16],
                    warm[:, 0:8],
                    start=True,
                    stop=True,
                )

            bank_ctr = [0]

            def load_chunk(img_rep, sp):
                """Emit chunk DMAs; last chunk of an image loads the QS+1
                zero row, others load QS rows (the q+1 read of the last
                group crosses into the NEXT chunk's row 0)."""
                img = img_rep % IMGS_PER_CORE
                q0 = QS * sp
                five = (img_rep * NS + sp) not in (0, 5, 10)
                lastsp = sp == NS - 1
                rows = QS + 1 if lastsp else QS
                xt = xpool.tile([128, QS + 1, W], BF16, tag="x")
                xt2 = None
                if five:
                    xt2 = xpool.tile([128, QS, W], BF16, tag="x2")
                    nc.sync.dma_start(
                        out=xt2, in_=x2_ext[:][img][:, q0 : q0 + QS, :]
                    )
                if img_rep == 0 and sp == 0:
                    nc.sync.dma_start(
                        out=xt[:, 0:3, :],
                        in_=x_ext[:][img][:, q0 : q0 + 3, :],
                    )
                    nc.sync.dma_start(
                        out=xt[:, 3:rows, :],
                        in_=x_ext[:][img][:, q0 + 3 : q0 + rows, :],
                    )
                else:
                    nc.sync.dma_start(
                        out=xt[:, 0:rows, :],
                        in_=x_ext[:][img][:, q0 : q0 + rows, :],
                    )
                return xt, xt2, five

            nreps = IMGS_PER_CORE * repeat
            cur = load_chunk(0, 0)
            for img_rep in range(nreps):
                img = img_rep % IMGS_PER_CORE
                for sp in range(NS):
                    q0 = QS * sp
                    xt, xt2, five = cur
                    # prefetch next chunk BEFORE this chunk's out-DMA so the
                    # cross-chunk q+1 read can't deadlock behind it
                    if sp < NS - 1:
                        nxt = load_chunk(img_rep, sp + 1)
                    elif img_rep < nreps - 1:
                        nxt = load_chunk(img_rep + 1, 0)
                    else:
                        nxt = None
                    xt_nx = nxt[0] if (nxt is not None and sp < NS - 1) else xt
                    ydst = y_ext[:][img].rearrange("d m q w -> (d m) q w")
                    stage = opool.tile([128, QS, OW], BF16)
                    tail = img_rep == IMGS_PER_CORE * repeat - 1 and sp == NS - 1
                    ng = 1 if tail else 2  # groups per plane (tail: finer)
                    for u in range(QS // ng):
                        plane = pspool.tile(
                            [128, 2, 256], F32, tag=f"bk{bank_ctr[0] % 8}"
                        )
                        bank_ctr[0] += 1
                        for qq in range(ng):
                            ql = ng * u + qq  # local group index in chunk
                            def rhs(offrow, delta):
                                if offrow == QS and sp < NS - 1:
                                    return xt_nx[:, 0, delta : delta + OW]
                                return xt[:, offrow, delta : delta + OW]

                            if five:
                                # taps 0-2: A deltas @q; tap 3: off1 d0 @q+1;
                                # tap 6: copy2 covers (rho>=4, delta in {1,2})
                                for tap in range(4):
                                    off = tap // 3
                                    delta = tap % 3
                                    nc.tensor.matmul(
                                        plane[:, qq, 0:OW],
                                        w_sb[:, tap, :],
                                        rhs(ql + off, delta),
                                        start=(tap == 0),
                                        stop=False,
                                    )
                                nc.tensor.matmul(
                                    plane[:, qq, 0:OW],
                                    w_sb[:, 6, :],
                                    xt2[:, ql, 0:OW],
                                    start=False,
                                    stop=True,
                                )
                            else:
                                for off in range(2):
                                    for delta in range(3):
                                        tap = 3 * off + delta
                                        nc.tensor.matmul(
                                            plane[:, qq, 0:OW],
                                            w_sb[:, tap, :],
                                            rhs(ql + off, delta),
                                            start=(tap == 0),
                                            stop=(tap == 5),
                                        )
                        nc.scalar.activation(
                            out=stage[:, ng * u : ng * u + ng, :],
                            in_=plane[:, 0:ng, 0:OW],
                            func=LRELU,
                            bias=bias_half,
                            scale=0.5,
                            alpha=0.01,
                        )
                        if tail:
                            nc.sync.dma_start(
                                out=ydst[:, q0 + ng * u : q0 + ng * u + ng, :],
                                in_=stage[:, ng * u : ng * u + ng, :],
                            )
                    if not tail:
                        nc.sync.dma_start(
                            out=ydst[:, q0 : q0 + QS, :], in_=stage
                        )
                    cur = nxt
    nc.compile()
    return nc


_CACHE = {}


def _get_nc(repeat=1):
    key = f"nc{repeat}"
    if key not in _CACHE:
        _CACHE[key] = build_nc(repeat)
    return _CACHE[key]


def make_w6_biasr(weight, bias):
    import ml_dtypes

    weight = np.asarray(weight, dtype=np.float32)
    bias = np.asarray(bias, dtype=np.float32)
    w6 = np.zeros((128, 7, 128), dtype=np.float32)
    for off in range(2):
        for delta in range(3):
            tap = 3 * off + delta
            for b in range(4):
                rho = 4 * off + b
                for d in range(4):
                    kh = rho - d
                    if 0 <= kh <= 2:
                        w6[
                            32 * b : 32 * b + 32,
                            tap,
                            32 * d : 32 * d + 32,
                        ] = weight[:, :, kh, delta].T
    # tap 6 = copy2 slots (rho, delta) = [(4,1),(4,2),(5,1),(5,2)]
    for s, (rho, delta) in enumerate([(4, 1), (4, 2), (5, 1), (5, 2)]):
        for d in range(4):
            kh = rho - d
            if 0 <= kh <= 2:
                w6[32 * s : 32 * s + 32, 6, 32 * d : 32 * d + 32] = (
                    weight[:, :, kh, delta].T
                )
    w6 = np.ascontiguousarray(w6.astype(ml_dtypes.bfloat16))
    biasr = np.ascontiguousarray(np.tile(bias * 0.5, 4).astype(np.float32))
    return w6, biasr


def make_xr(x):
    """xr[n, 32b+c, rr, w] = x[n, c, 4rr+b, w]; rr=64 is zeros."""
    import ml_dtypes

    x = np.asarray(x, dtype=np.float32).astype(ml_dtypes.bfloat16)
    n = x.shape[0]
    v = x.reshape(n, C, 64, 4, W).transpose(0, 3, 1, 2, 4).reshape(n, 128, 64, W)
    out = np.zeros((n, 128, 65, W), dtype=x.dtype)
    out[:, :, 0:64, :] = v
    return np.ascontiguousarray(out)


def make_xc2(x):
    """xc2[n, 32s+c, rr, w] = x[n, c, 4rr+4+s//2, w + 1+s%2] (zero-pad OOB)."""
    import ml_dtypes

    x = np.asarray(x, dtype=np.float32).astype(ml_dtypes.bfloat16)
    n = x.shape[0]
    out = np.zeros((n, 128, 64, W), dtype=x.dtype)
    for s, (rho, delta) in enumerate([(4, 1), (4, 2), (5, 1), (5, 2)]):
        rows = np.zeros((n, C, 64, W), dtype=x.dtype)
        hs = np.arange(64) * 4 + rho
        valid = hs < H
        src_rows = x[:, :, hs[valid], :]
        rows[:, :, valid, : W - delta] = src_rows[:, :, :, delta:]
        out[:, 32 * s : 32 * s + 32, :, :] = rows
    return np.ascontiguousarray(out)


def make_in_maps(x, weight, bias):
    xr = make_xr(x)
    xc2 = make_xc2(x)
    w6, biasr = make_w6_biasr(weight, bias)
    return [
        {
            "xr": xr[IMGS_PER_CORE * i : IMGS_PER_CORE * (i + 1)],
            "xc2": xc2[IMGS_PER_CORE * i : IMGS_PER_CORE * (i + 1)],
            "w6": w6,
            "biasr": biasr,
        }
        for i in range(N_CORES)
    ]


def postprocess_y(y_banded):
    """[n, 4, C, 64, OW] bf16 (row = 4q+d) -> [n, C, 254, 254] f32."""
    y = np.asarray(y_banded).astype(np.float32)
    n = y.shape[0]
    y = y.transpose(0, 2, 3, 1, 4).reshape(n, C, 256, OW)
    return np.ascontiguousarray(y[:, :, 0:OH, :])


def kernel(x, weight, bias):
    in_maps = make_in_maps(x, weight, bias)
    nc = _get_nc()
    res = None
    for attempt in range(4):
        try:
            res = run_bass_kernel_spmd(
                nc, in_maps, core_ids=list(range(N_CORES))
            )
            break
        except Exception:
            # transient device fault (axon terminal resets itself in ~2 min);
            # a wedge can persist through one reset window, so retry a few
            # times with full waits
            if attempt == 3:
                raise
            import time as _time

            _time.sleep(150)
    y = np.concatenate(
        [postprocess_y(res.results[i]["y"]) for i in range(N_CORES)], axis=0
    )
    return y


# revision 21
# speedup vs baseline: 6.7463x; 1.0031x over previous
"""v11: cross-group offset matmuls -- 6 taps per 4-row group, ONE x copy.

Single row-phase copy: xr[img, 32b+c, rr, w] = x[c, 4rr+b, w] (rr=64 zeros).
Reading free row q gives partition block b the image row 4q+b (rho=b);
reading free row q+1 gives rho=4+b. Weight for (offset off, shift delta):
  w6[32b+c, 3*off+delta, 32d+m] = w[m, c, rho-d, delta],  rho = 4*off+b,
  nonzero wherever 0 <= rho-d <= 2.
One matmul then serves ALL (d, kh) pairs with d+kh = rho at once; the six
(off, delta) matmuls cover all 36 (d, kh, delta) terms exactly once:
  out[(d,m), n] = sum_{kh,kw} w[m,c,kh,kw] x[c, 4q+d+kh, n+kw].
6 matmuls x N=254 per 4 output rows (1.5 rows/pixel) vs 9 for the
block-diagonal form. PE ~81 us, input DMA ~28 us (one copy).

y is row-interleaved [img, d, C, 64, OW] (row = 4q+d; rows 254/255 junk);
host reassembles.
"""

import sys

if "/opt/trn_rl_repo" not in sys.path:
    sys.path.insert(0, "/opt/trn_rl_repo")

import numpy as np

import concourse.bass as bass
import concourse.tile as tile
from concourse import bacc
from concourse import mybir
from concourse.bass_utils import run_bass_kernel_spmd

N_CORES = 8
IMGS_PER_CORE = 2
C = 32
H = 256
W = 256
OH = 254
OW = 254
NS = 16          # group-chunks per image
QS = 64 // NS    # groups per chunk (8) = 32 output rows
F32 = mybir.dt.float32
BF16 = mybir.dt.bfloat16
LRELU = mybir.ActivationFunctionType.Lrelu


def build_nc(repeat=1):
    nc = bacc.Bacc()
    x_ext = nc.declare_dram_parameter(
        "xr", [IMGS_PER_CORE, 128, 65, W], BF16, isOutput=False
    )
    x2_ext = nc.declare_dram_parameter(
        "xc2", [IMGS_PER_CORE, 128, 64, W], BF16, isOutput=False
    )
    w_ext = nc.declare_dram_parameter("w6", [128, 7, 128], BF16, isOutput=False)
    b_ext = nc.declare_dram_parameter("biasr", [128], F32, isOutput=False)
    # y[img, d, m, q, w] = out[m, 4q+d, w]; rows 254/255 are junk pad
    y_ext = nc.declare_dram_parameter(
        "y", [IMGS_PER_CORE, 4, C, 64, OW], BF16, isOutput=True
    )

    with tile.TileContext(nc) as tc:
        with (
            tc.tile_pool(name="xp", bufs=3) as xpool,
            tc.tile_pool(name="const", bufs=1) as cpool,
            tc.tile_pool(name="ps", bufs=1, space="PSUM") as pspool,
            tc.tile_pool(name="outp", bufs=3) as opool,
        ):
            w_sb = cpool.tile([128, 7, 128], BF16)
            bias_half = cpool.tile([128, 1], F32)
            nc.sync.dma_start(out=w_sb, in_=w_ext[:])
            nc.sync.dma_start(out=bias_half, in_=b_ext[:].unsqueeze(1))

            # PE p-state warmup: dead N=8 matmuls during the initial DMA
            # wait keep the PE busy so real matmuls start at 2.4 GHz.
            warm = cpool.tile([128, 16], BF16)
            nc.gpsimd.memset(warm, 0.0)
            wplane = pspool.tile([128, 2, 256], F32, tag="bk7")
            for _ in range(530):
                nc.tensor.matmul(
                    wplane[0:16, 0, 0:8],
                    warm[:, 0:16],
                    warm[:, 0:8],
                    start=True,
                    stop=True,
                )

            bank_ctr = [0]

            def load_chunk(img_rep, sp):
                """Emit chunk DMAs; last chunk of an image loads the QS+1
                zero row, others load QS rows (the q+1 read of the last
                group crosses into the NEXT chunk's row 0)."""
                img = img_rep % IMGS_PER_CORE
                q0 = QS * sp
                five = (img_rep * NS + sp) % 5 != 0
                lastsp = sp == NS - 1
                rows = QS + 1 if lastsp else QS
                xt = xpool.tile([128, QS + 1, W], BF16, tag="x")
                xt2 = None
                if five:
                    xt2 = xpool.tile([128, QS, W], BF16, tag="x2")
                    nc.sync.dma_start(
                        out=xt2, in_=x2_ext[:][img][:, q0 : q0 + QS, :]
                    )
                if img_rep == 0 and sp == 0:
                    nc.sync.dma_start(
                        out=xt[:, 0:3, :],
                        in_=x_ext[:][img][:, q0 : q0 + 3, :],
                    )
                    nc.sync.dma_start(
                        out=xt[:, 3:rows, :],
                        in_=x_ext[:][img][:, q0 + 3 : q0 + rows, :],
                    )
                else:
                    nc.sync.dma_start(
                        out=xt[:, 0:rows, :],
                        in_=x_ext[:][img][:, q0 : q0 + rows, :],
                    )
                return xt, xt2, five

            nreps = IMGS_PER_CORE * repeat
            cur = load_chunk(0, 0)
            for img_rep in range(nreps):
                img = img_rep % IMGS_PER_CORE
                for sp in range(NS):
                    q0 = QS * sp
                    xt, xt2, five = cur
                    # prefetch next chunk BEFORE this chunk's out-DMA so the
                    # cross-chunk q+1 read can't deadlock behind it
                    if sp < NS - 1:
                        nxt = load_chunk(img_rep, sp + 1)
                    elif img_rep < nreps - 1:
                        nxt = load_chunk(img_rep + 1, 0)
                    else:
                        nxt = None
                    xt_nx = nxt[0] if (nxt is not None and sp < NS - 1) else xt
                    ydst = y_ext[:][img].rearrange("d m q w -> (d m) q w")
                    stage = opool.tile([128, QS, OW], BF16)
                    tail = img_rep == IMGS_PER_CORE * repeat - 1 and sp == NS - 1
                    ng = 1 if tail else 2  # groups per plane (tail: finer)
                    for u in range(QS // ng):
                        plane = pspool.tile(
                            [128, 2, 256], F32, tag=f"bk{bank_ctr[0] % 8}"
                        )
                        bank_ctr[0] += 1
                        for qq in range(ng):
                            ql = ng * u + qq  # local group index in chunk
                            def rhs(offrow, delta):
                                if offrow == QS and sp < NS - 1:
                                    return xt_nx[:, 0, delta : delta + OW]
                                return xt[:, offrow, delta : delta + OW]

                            if five:
                                # taps 0-2: A deltas @q; tap 3: off1 d0 @q+1;
                                # tap 6: copy2 covers (rho>=4, delta in {1,2})
                                for tap in range(4):
                                    off = tap // 3
                                    delta = tap % 3
                                    nc.tensor.matmul(
                                        plane[:, qq, 0:OW],
                                        w_sb[:, tap, :],
                                        rhs(ql + off, delta),
                                        start=(tap == 0),
                                        stop=False,
                                    )
                                nc.tensor.matmul(
                                    plane[:, qq, 0:OW],
                                    w_sb[:, 6, :],
                                    xt2[:, ql, 0:OW],
                                    start=False,
                                    stop=True,
                                )
                            else:
                                for off in range(2):
                                    for delta in range(3):
                                        tap = 3 * off + delta
                                        nc.tensor.matmul(
                                            plane[:, qq, 0:OW],
                                            w_sb[:, tap, :],
                                            rhs(ql + off, delta),
                                            start=(tap == 0),
                                            stop=(tap == 5),
                                        )
                        nc.scalar.activation(
                            out=stage[:, ng * u : ng * u + ng, :],
                            in_=plane[:, 0:ng, 0:OW],
                            func=LRELU,
                            bias=bias_half,
                            scale=0.5,
                            alpha=0.01,
                        )
                        if tail:
                            nc.sync.dma_start(
                                out=ydst[:, q0 + ng * u : q0 + ng * u + ng, :],
                                in_=stage[:, ng * u : ng * u + ng, :],
                            )
                    if not tail:
                        nc.sync.dma_start(
                            out=ydst[:, q0 : q0 + QS, :], in_=stage
                        )
                    cur = nxt
    nc.compile()
    return nc


_CACHE = {}


def _get_nc(repeat=1):
    key = f"nc{repeat}"
    if key not in _CACHE:
        _CACHE[key] = build_nc(repeat)
    return _CACHE[key]


def make_w6_biasr(weight, bias):
    import ml_dtypes

    weight = np.asarray(weight, dtype=np.float32)
    bias = np.asarray(bias, dtype=np.float32)
    w6 = np.zeros((128, 7, 128), dtype=np.float32)
    for off in range(2):
        for delta in range(3):
            tap = 3 * off + delta
            for b in range(4):
                rho = 4 * off + b
                for d in range(4):
                    kh = rho - d
                    if 0 <= kh <= 2:
                        w6[
                            32 * b : 32 * b + 32,
                            tap,
                            32 * d : 32 * d + 32,
                        ] = weight[:, :, kh, delta].T
    # tap 6 = copy2 slots (rho, delta) = [(4,1),(4,2),(5,1),(5,2)]
    for s, (rho, delta) in enumerate([(4, 1), (4, 2), (5, 1), (5, 2)]):
        for d in range(4):
            kh = rho - d
            if 0 <= kh <= 2:
                w6[32 * s : 32 * s + 32, 6, 32 * d : 32 * d + 32] = (
                    weight[:, :, kh, delta].T
                )
    w6 = np.ascontiguousarray(w6.astype(ml_dtypes.bfloat16))
    biasr = np.ascontiguousarray(np.tile(bias * 0.5, 4).astype(np.float32))
    return w6, biasr


def make_xr(x):
    """xr[n, 32b+c, rr, w] = x[n, c, 4rr+b, w]; rr=64 is zeros."""
    import ml_dtypes

    x = np.asarray(x, dtype=np.float32).astype(ml_dtypes.bfloat16)
    n = x.shape[0]
    v = x.reshape(n, C, 64, 4, W).transpose(0, 3, 1, 2, 4).reshape(n, 128, 64, W)
    out = np.zeros((n, 128, 65, W), dtype=x.dtype)
    out[:, :, 0:64, :] = v
    return np.ascontiguousarray(out)


def make_xc2(x):
    """xc2[n, 32s+c, rr, w] = x[n, c, 4rr+4+s//2, w + 1+s%2] (zero-pad OOB)."""
    import ml_dtypes

    x = np.asarray(x, dtype=np.float32).astype(ml_dtypes.bfloat16)
    n = x.shape[0]
    out = np.zeros((n, 128, 64, W), dtype=x.dtype)
    for s, (rho, delta) in enumerate([(4, 1), (4, 2), (5, 1), (5, 2)]):
        rows = np.zeros((n, C, 64, W), dtype=x.dtype)
        hs = np.arange(64) * 4 + rho
        valid = hs < H
        src_rows = x[:, :, hs[valid], :]
        rows[:, :, valid, : W - delta] = src_rows[:, :, :, delta:]
        out[:, 32 * s : 32 * s + 32, :, :] = rows
    return np.ascontiguousarray(out)


def make_in_maps(x, weight, bias):
    xr = make_xr(x)
    xc2 = make_xc2(x)
    w6, biasr = make_w6_biasr(weight, bias)
    return [
        {
            "xr": xr[IMGS_PER_CORE * i : IMGS_PER_CORE * (i + 1)],
            "xc2": xc2[IMGS_PER_CORE * i : IMGS_PER_CORE * (i + 1)],
            "w6": w6,
            "biasr": biasr,
        }
        for i in range(N_CORES)
    ]


def postprocess_y(y_banded):
    """[n, 4, C, 64, OW] bf16 (row = 4q+d) -> [n, C, 254, 254] f32."""
    y = np.asarray(y_banded).astype(np.float32)
    n = y.shape[0]
    y = y.transpose(0, 2, 3, 1, 4).reshape(n, C, 256, OW)
    return np.ascontiguousarray(y[:, :, 0:OH, :])


def kernel(x, weight, bias):
    in_maps = make_in_maps(x, weight, bias)
    nc = _get_nc()
    res = None
    for attempt in range(4):
        try:
            res = run_bass_kernel_spmd(
                nc, in_maps, core_ids=list(range(N_CORES))
            )
            break
        except Exception:
            # transient device fault (axon terminal resets itself in ~2 min);
            # a wedge can persist through one reset window, so retry a few
            # times with full waits
            if attempt == 3:
                raise
            import time as _time

            _time.sleep(150)
    y = np.concatenate(
        [postprocess_y(res.results[i]["y"]) for i in range(N_CORES)], axis=0
    )
    return y
